# revision 2
# baseline (speedup 1.0000x reference)
"""Trainium2 Bass kernel for nn_AssociationLayer (sparse-attention transformer block).

v2: data-parallel over batch (2 samples/core, 8 cores), optimized against the
TRN2 cost model:
  - few, large DMAs (HWDGE is a serialized ~625ns/instruction resource)
  - LN transposes on the PE (identity matmul) with Pool-engine PSUM drains
  - ACT calls batched to 1024-wide; all exp/ln grouped before all gelu
  - key-mask baked into kt host-side (NEG rows); exp bias = constant SHIFT
  - kt tile loads pair-merged and issued inside tc.If (skipped => no cost)
  - per-sample phases interleaved so PE/ACT/DVE/Pool overlap across samples

Attention math (same as baseline, validated vs reference): with nrc = n1*n2,
  rows i <  nrc: softmax over keys j < nrc of (q_i.k_j/sqrt(D) + K[i,j]) @ v
  rows i >= nrc: uniform attention = mean over ALL keys of v
Scores are computed transposed (S^T[j,i]) so masked keys ride the kt rows and
exp(S^T) feeds the AV matmul as lhsT. Row sums come from a ones-column in va;
1/s is exp(-ln(s)) on ACT.
"""

import numpy as np

B, N, C = 16, 1024, 256
H, D = 4, 64
NCORES = 8
SPC = 2
P = 128
NT = N // P
ICW = 512
NEG = -1.0e10
SHIFT = -12.0
EPS = 1e-5

# bf16 weight blob layout (per partition, in order):
#   [c2=0: wqk(512) wv(256) proj(256) fc1(1024)] [c2=1: same] [ident(128)] [fc2 8*256]
WB_C2 = 2 * C + C + C + 4 * C  # 2048
O_WQK, O_WV, O_PROJ, O_FC1 = 0, 2 * C, 3 * C, 4 * C
O_ID = 2 * WB_C2
O_FC2 = O_ID + P
O_ROWS = O_FC2 + 8 * C  # bv_row, pb_row, f2b_row (bf16)
WB = O_ROWS + 3 * C
# f32 blob: bqk(4) bv(2) bf1(8) iota(8) sel(256)
F_BQK, F_BV, F_BF1, F_IOTA, F_SEL = 0, 4, 6, 14, 22
F32W = 22 + C


def _build(R_max, C_max, nz):
    """nz: dict of nonzero-bias flags {'bqk','bv','pb','f2b'}."""
    import concourse.bass as bass
    import concourse.mybir as mybir
    import concourse.tile as tile
    from concourse import bacc

    f32 = mybir.dt.float32
    bf16 = mybir.dt.bfloat16
    Alu = mybir.AluOpType
    Act = mybir.ActivationFunctionType

    # Force Exp/Ln into the combined natural_log_exp set so the greedy
    # table-load pass doesn't ping-pong (each reload costs ~2.7us on ACT).
    import concourse.hw_specs as hw_specs
    if not getattr(bacc, "_act_tables_patched", False):
        _orig_get_tables = hw_specs.get_activation_tables

        def _patched_tables(arch):
            tabs = dict(_orig_get_tables(arch))
            for nm in list(tabs.keys()):
                if nm != "natural_log_exp_and_others":
                    tabs[nm] = set(tabs[nm]) - {Act.Exp, Act.Ln}
            return tabs

        bacc.get_activation_tables = _patched_tables
        bacc._act_tables_patched = True

    nc = bacc.Bacc()

    NPAIR = (R_max + 1) // 2
    wb_ext = nc.declare_dram_parameter("wb", [P, WB // 2], f32, isOutput=False)
    cb_ext = nc.declare_dram_parameter("cb", [P, F32W], f32, isOutput=False)
    meta_ext = nc.declare_dram_parameter("meta", [1, 8], mybir.dt.int32, isOutput=False)
    big_ext = nc.declare_dram_parameter("big", [SPC, N * N // 2 + N * C], f32, isOutput=False)
    out_ext = nc.declare_dram_parameter("out", [SPC, N, C], f32, isOutput=True)

    kt_ext = big_ext[:, 0:N * N // 2].bitcast(bf16).rearrange("s (j i) -> s j i", i=N)
    x_ext = big_ext[:, N * N // 2:].rearrange("s (i c) -> s i c", c=C)

    with tile.TileContext(nc) as tc:
        with (
            tc.tile_pool(name="singles", bufs=1) as singles,
            tc.tile_pool(name="big", bufs=2) as big,
            tc.tile_pool(name="hnp", bufs=4) as hnp,
            tc.tile_pool(name="epi", bufs=1) as epi,
            tc.tile_pool(name="work", bufs=2) as work,
            tc.tile_pool(name="stats", bufs=4) as stats,
            tc.tile_pool(name="ktp", bufs=2) as ktp,
            tc.tile_pool(name="ptp", bufs=5) as ptp,
            tc.tile_pool(name="mtp", bufs=1) as mtp,
            tc.tile_pool(name="psw", bufs=2, space="PSUM") as psw,
            tc.tile_pool(name="psacc", bufs=1, space="PSUM") as psacc,
        ):
            # ---------- constants ----------
            meta_sb = singles.tile([1, 8], mybir.dt.int32, tag="meta")
            nc.sync.dma_start(out=meta_sb[:], in_=meta_ext[:])
            cb_sb = singles.tile([P, F32W], f32, tag="cb")
            nc.sync.dma_start(out=cb_sb[:], in_=cb_ext[:])
            wb_sb = singles.tile([P, WB], bf16, tag="wb")
            wb_bf = wb_ext[:].bitcast(bf16)

            def wqk(c2):  # [P, 512]
                o = c2 * WB_C2 + O_WQK
                return wb_sb[:, o:o + 2 * C]

            def wv(c2):
                o = c2 * WB_C2 + O_WV
                return wb_sb[:, o:o + C]

            def wproj(c2):
                o = c2 * WB_C2 + O_PROJ
                return wb_sb[:, o:o + C]

            def wfc1(c2):
                o = c2 * WB_C2 + O_FC1
                return wb_sb[:, o:o + 4 * C]

            id_sb = wb_sb[:, O_ID:O_ID + P]

            def wfc2(r):
                o = O_FC2 + r * C
                return wb_sb[:, o:o + C]

            bvrow_sb = wb_sb[0:1, O_ROWS:O_ROWS + C]
            pbrow_sb = wb_sb[0:1, O_ROWS + C:O_ROWS + 2 * C]
            f2brow_sb = wb_sb[0:1, O_ROWS + 2 * C:O_ROWS + 3 * C]

            bqk_sb = cb_sb[:, F_BQK:F_BQK + 4]
            bf1_sb = cb_sb[:, F_BF1:F_BF1 + 8]
            iota_sb = cb_sb[:, F_IOTA:F_IOTA + NT]
            sel_sb = cb_sb[:, F_SEL:F_SEL + C]

            eps_sb = singles.tile([P, 1], f32, tag="eps")
            nc.gpsimd.memset(eps_sb[:], EPS)
            shift_sb = singles.tile([P, 1], f32, tag="shift")
            nc.gpsimd.memset(shift_sb[:], SHIFT)
            ones1_sb = singles.tile([1, P], f32, tag="ones1")
            nc.gpsimd.memset(ones1_sb[:], 1.0)
            ones1_bf = singles.tile([1, P], bf16, tag="ones1bf")
            nc.gpsimd.memset(ones1_bf[:], 1.0)
            ucol_sb = singles.tile([P, 1], bf16, tag="ucol")
            nc.gpsimd.memset(ucol_sb[:], 1.0 / N)
            stage_sb = singles.tile([P, ICW], bf16, tag="stage")
            nc.gpsimd.memset(stage_sb[:], 1.0)
            selb_sb = singles.tile([P, C], bf16, tag="selb")
            nc.vector.tensor_copy(out=selb_sb[:], in_=sel_sb)
            zcol_sb = singles.tile([1, D + 1], bf16, tag="zcol")
            nc.gpsimd.memset(zcol_sb[:], 0.0)
            zrow_sb = singles.tile([1, ICW], bf16, tag="zrow")
            nc.gpsimd.memset(zrow_sb[:], 0.0)

            # per-slot persistents
            x_sb = [singles.tile([P, NT, C], f32, tag=f"x{s}", name=f"x{s}") for s in range(SPC)]
            mval = [singles.tile([P, NT], f32, tag=f"mval{s}", name=f"mval{s}") for s in range(SPC)]
            minv = [singles.tile([P, NT], f32, tag=f"minv{s}", name=f"minv{s}") for s in range(SPC)]
            ub_sb = [singles.tile([P, C], f32, tag=f"ub{s}", name=f"ub{s}") for s in range(SPC)]
            qkT = [singles.tile([P, 4, N], bf16, tag=f"qkT{s}", name=f"qkT{s}") for s in range(SPC)]
            va = [singles.tile([P, NT, H, D + 1], bf16, tag=f"va{s}", name=f"va{s}") for s in range(SPC)]

            nc.sync.dma_start(out=wb_sb[:], in_=wb_bf)
            for s in range(SPC):
                nc.sync.dma_start(out=x_sb[s][:], in_=x_ext[s].rearrange("(t p) c -> p t c", p=P))

            # masks from nrc (meta: [R1a,R2a,R1b,R2b, nrc_a, nrc_b, 0, 0])
            nrcf_sb = singles.tile([P, SPC], f32, tag="nrcf")
            nc.sync.dma_start(
                out=nrcf_sb[:],
                in_=meta_ext[:, 4:6].bitcast(f32).to_broadcast((P, SPC)))
            Rv, R2v, Lpv = [None] * SPC, [None] * SPC, [None] * SPC
            for s in range(SPC):
                nc.vector.tensor_scalar(out=mval[s][:], in0=iota_sb, scalar1=nrcf_sb[:, s:s + 1],
                                        scalar2=None, op0=Alu.is_lt)
                nc.vector.tensor_scalar(out=minv[s][:], in0=mval[s][:], scalar1=-1.0,
                                        scalar2=1.0, op0=Alu.mult, op1=Alu.add)
                Rv[s] = nc.values_load(meta_sb[0:1, 2 * s:2 * s + 1], min_val=0, max_val=NT,
                                       skip_runtime_bounds_check=True)
                R2v[s] = nc.values_load(meta_sb[0:1, 2 * s + 1:2 * s + 2], min_val=0, max_val=NT,
                                        skip_runtime_bounds_check=True)
                Lpv[s] = nc.values_load(meta_sb[0:1, 6 + s:7 + s], min_val=0, max_val=NPAIR - 1,
                                        skip_runtime_bounds_check=True)

            # ---------- helpers ----------
            def ln_transposed(src3, hT, mid_thunk=None):
                """LayerNorm over free dim C of all NT tiles of src3 (row layout)
                -> hT [P, 2, N] bf16 (transposed) via merged xbar DMA transposes.
                mid_thunk (if given) is emitted after the first half of tiles."""
                mv8 = stats.tile([P, 2, NT], f32, tag="mv8", name="mv8")
                rstd8 = stats.tile([P, NT], f32, tag="rstd8", name="rstd8")
                for g in range(2):
                    for tt in range(4):
                        t = g * 4 + tt
                        st6 = stats.tile([P, 6], f32, tag="st6")
                        nc.vector.bn_stats(out=st6[:], in_=src3[:, t, :])
                        nc.vector.bn_aggr(out=mv8[:, :, t], in_=st6[:])
                    lnv4 = stats.tile([P, 4], f32, tag="lnv4")
                    nc.scalar.activation(out=lnv4[:], in_=mv8[:, 1, g * 4:g * 4 + 4],
                                         func=Act.Ln, bias=eps_sb[:], scale=1.0)
                    nc.scalar.activation(out=rstd8[:, g * 4:g * 4 + 4], in_=lnv4[:],
                                         func=Act.Exp, bias=0.0, scale=-0.5)
                    for tt in range(4):
                        t = g * 4 + tt
                        hn = hnp.tile([P, C], bf16, tag="hn", name="hn")
                        nc.vector.tensor_scalar(out=hn[:], in0=src3[:, t, :],
                                                scalar1=mv8[:, 0, t:t + 1],
                                                scalar2=rstd8[:, t:t + 1],
                                                op0=Alu.subtract, op1=Alu.mult)
                        nc.sync.dma_start_transpose(out=hT[:, 0:2, t * P:(t + 1) * P],
                                                    in_=hn[:])
                    if g == 0 and mid_thunk is not None:
                        mid_thunk()

            def qkT_chunk(s, hT, icq):
                ps = psw.tile([P, 2, ICW], f32, tag="w", name="psqk")
                for half in range(2):  # q rows then k rows
                    for rr in range(2):
                        r = half * 2 + rr
                        for c2 in range(2):
                            nc.tensor.matmul(ps[:, rr, :], lhsT=wqk(c2)[:, r * P:(r + 1) * P],
                                             rhs=hT[:, c2, icq * ICW:(icq + 1) * ICW],
                                             start=(c2 == 0), stop=(c2 == 1))
                    for rr in range(2):
                        r = half * 2 + rr
                        eng = nc.vector if rr == 0 else nc.any
                        if nz["bqk"]:
                            eng.tensor_scalar(out=qkT[s][:, r, icq * ICW:(icq + 1) * ICW],
                                              in0=ps[:, rr, :], scalar1=bqk_sb[:, r:r + 1],
                                              scalar2=None, op0=Alu.add)
                        else:
                            eng.tensor_copy(out=qkT[s][:, r, icq * ICW:(icq + 1) * ICW],
                                            in_=ps[:, rr, :])

            def va_block(s, hT):
                nc.gpsimd.memset(va[s][:, :, :, D:D + 1], 1.0)
                for t in range(NT):
                    psv = psw.tile([P, 2, ICW], f32, tag="w", name="psv")
                    for c2 in range(2):
                        nc.tensor.matmul(psv[:, 0, 0:C], lhsT=hT[:, c2, t * P:(t + 1) * P],
                                         rhs=wv(c2), start=(c2 == 0),
                                         stop=(c2 == 1 and not nz["bv"]))
                    if nz["bv"]:
                        nc.tensor.matmul(psv[:, 0, 0:C], lhsT=ones1_bf[:], rhs=bvrow_sb,
                                         start=False, stop=True)
                    eng = nc.vector if t % 2 == 0 else nc.any
                    eng.tensor_copy(out=va[s][:, t, :, 0:D],
                                    in_=psv[:, 0, 0:C].rearrange("p (h d) -> p h d", h=H))

            def uniform_block(s):
                # u = mean_v @ projT (+pb); bv already folded into va rows.
                psmv = psw.tile([P, 2, ICW], f32, tag="w", name="psmv")
                for t in range(NT):
                    nc.tensor.matmul(psmv[0:1, 0, 0:H * (D + 1)], lhsT=ucol_sb[:],
                                     rhs=va[s][:, t, :, :], start=(t == 0), stop=(t == NT - 1))
                u_tmp = work.tile([1, C], f32, tag="utmp")
                for h in range(H):
                    nc.any.tensor_copy(out=u_tmp[0:1, h * D:(h + 1) * D],
                                       in_=psmv[0:1, 0, h * (D + 1):h * (D + 1) + D])
                mvT = work.tile([P, 2], bf16, tag="mvT")
                for c2 in range(2):
                    pst = psw.tile([P, 2, ICW], f32, tag="w", name="pst")
                    nc.tensor.matmul(pst[:, 0, 0:1], lhsT=u_tmp[0:1, c2 * P:(c2 + 1) * P],
                                     rhs=ones1_sb[0:1, 0:1], start=True, stop=True)
                    nc.any.tensor_copy(out=mvT[:, c2:c2 + 1], in_=pst[:, 0, 0:1])
                psu = psw.tile([P, 2, ICW], f32, tag="w", name="psu")
                for c2 in range(2):
                    nc.tensor.matmul(psu[0:1, 0, 0:C], lhsT=mvT[:, c2:c2 + 1], rhs=wproj(c2),
                                     start=(c2 == 0), stop=(c2 == 1 and not nz["pb"]))
                if nz["pb"]:
                    nc.tensor.matmul(psu[0:1, 0, 0:C], lhsT=ones1_bf[0:1, 0:1], rhs=pbrow_sb,
                                     start=False, stop=True)
                u_row = work.tile([1, C], bf16, tag="urow")
                nc.vector.tensor_copy(out=u_row[:], in_=psu[0:1, 0, 0:C])
                psub = psw.tile([P, 2, ICW], f32, tag="w", name="psub")
                nc.tensor.matmul(psub[:, 0, 0:C], lhsT=ones1_bf[:], rhs=u_row[:], start=True, stop=True)
                nc.any.tensor_copy(out=ub_sb[s][:], in_=psub[:, 0, 0:C])
                # dense pre-pass: x2 = x + u * (1 - m)
                for g in range(NT):
                    nc.vector.scalar_tensor_tensor(out=x_sb[s][:, g, :], in0=ub_sb[s][:],
                                                   scalar=minv[s][:, g:g + 1],
                                                   in1=x_sb[s][:, g, :],
                                                   op0=Alu.mult, op1=Alu.add)

            # ---------- attention ----------
            def kt_prefetch(s, ic, Rcond, first):
                ktt = ktp.tile([P, R_max, ICW], bf16, tag="ktt", name=f"ktt{s}_{ic}")
                for pr in range(NPAIR):
                    j0 = 2 * pr
                    npair = min(2, R_max - j0)
                    src = kt_ext[s, j0 * P:(j0 + npair) * P, ic * ICW:(ic + 1) * ICW]
                    src = src.rearrange("(two p) i -> p two i", p=P)

                    def emit(src=src, ktt=ktt, j0=j0, npair=npair):
                        nc.sync.dma_start(out=ktt[:, j0:j0 + npair, :], in_=src)

                    if first and pr == 0:
                        emit()
                    else:
                        with tc.If(Rcond > j0):
                            emit()
                return ktt

            def attn_avs(s, pr, pT, psav):
                for h in range(H):
                    for jj in range(min(2, R_max - 2 * pr)):
                        jt = 2 * pr + jj
                        nc.tensor.matmul(psav[h][:], lhsT=va[s][:, jt, h, :],
                                         rhs=pT[:, h, jj, :],
                                         start=(jt == 0), stop=False,
                                         skip_group_check=True)

            def attn_pair(s, ic, pr, ktt, pTs, psav):
                """scores + exp for pair pr; AV for pair pr-1 (SW pipeline)."""
                j0 = 2 * pr
                npair = min(2, R_max - j0)
                pT = ptp.tile([P, H, 2, ICW], bf16, tag="pT", name="pT")
                pTs.append(pT)
                for hp in range(2):  # head pairs
                    pss = [None, None]
                    for hh in range(2):
                        pss[hh] = psw.tile([P, 2, ICW], f32, tag="w", name=f"pss{hh}")
                        for jj in range(npair):
                            nc.tensor.matmul(pss[hh][:, jj, :], lhsT=id_sb,
                                             rhs=ktt[:, j0 + jj, :], start=True, stop=False)
                    for hh in range(2):
                        h = hp * 2 + hh
                        for jj in range(npair):
                            jt = j0 + jj
                            mo = (h % 2) * D
                            nc.tensor.matmul(pss[hh][:, jj, :],
                                             lhsT=qkT[s][mo:mo + D, 2 + h // 2, jt * P:(jt + 1) * P],
                                             rhs=qkT[s][mo:mo + D, h // 2, ic * ICW:(ic + 1) * ICW],
                                             start=False, stop=True)
                    for hh in range(2):
                        h = hp * 2 + hh
                        if npair == 2:
                            nc.scalar.activation(out=pT[:, h, :, :], in_=pss[hh][:],
                                                 func=Act.Exp, bias=shift_sb[:], scale=1.0)
                        else:
                            nc.scalar.activation(out=pT[:, h, 0, :], in_=pss[hh][:, 0, :],
                                                 func=Act.Exp, bias=shift_sb[:], scale=1.0)
                if pr > 0:
                    attn_avs(s, pr - 1, pTs[pr - 1], psav)

            def attn_tail(s, ic, psav, pTs):
                for pr in range(NPAIR):
                    with tc.If(Lpv[s] == pr):
                        attn_avs(s, pr, pTs[pr], psav)
                # close the (possibly branch-shortened) accumulation groups
                for h in range(H):
                    nc.tensor.matmul(psav[h][:], lhsT=zcol_sb[:], rhs=zrow_sb[:],
                                     start=False, stop=True, skip_group_check=True)
                # softmax denominators -> r = 1/s broadcast per head
                for h in range(H):
                    eng = nc.vector if h % 2 == 0 else nc.any
                    eng.tensor_copy(out=stage_sb[32 * h:32 * h + 1, :],
                                    in_=psav[h][D:D + 1, :])
                psr = psw.tile([P, 2, ICW], f32, tag="w", name="psr")
                for c2 in range(2):
                    nc.tensor.matmul(psr[:, c2, :], lhsT=selb_sb[:, c2 * P:(c2 + 1) * P],
                                     rhs=stage_sb[:], start=True, stop=True)
                lntmp = epi.tile([P, 2, ICW], f32, tag="lntmp", name="lntmp")
                nc.scalar.activation(out=lntmp[:], in_=psr[:], func=Act.Ln, bias=0.0, scale=1.0)
                r_sb = epi.tile([P, 2, ICW], f32, tag="rsb", name="rsb")
                nc.scalar.activation(out=r_sb[:], in_=lntmp[:], func=Act.Exp, bias=0.0, scale=-1.0)
                oT = epi.tile([P, 2, ICW], bf16, tag="oT", name="oT")
                for h in range(H):
                    mo = (h % 2) * D
                    nc.vector.tensor_tensor(out=oT[mo:mo + D, h // 2, :],
                                            in0=psav[h][0:D, :],
                                            in1=r_sb[mo:mo + D, h // 2, :], op=Alu.mult)
                for it in range(ICW // P):
                    g = ic * (ICW // P) + it
                    psp = psw.tile([P, 2, ICW], f32, tag="w", name="psp")
                    for c2 in range(2):
                        nc.tensor.matmul(psp[:, 0, 0:C], lhsT=oT[:, c2, it * P:(it + 1) * P],
                                         rhs=wproj(c2), start=(c2 == 0),
                                         stop=(c2 == 1 and not nz["pb"]))
                    if nz["pb"]:
                        nc.tensor.matmul(psp[:, 0, 0:C], lhsT=ones1_bf[:], rhs=pbrow_sb,
                                         start=False, stop=True)
                    nc.vector.scalar_tensor_tensor(out=x_sb[s][:, g, :], in0=psp[:, 0, 0:C],
                                                   scalar=mval[s][:, g:g + 1],
                                                   in1=x_sb[s][:, g, :],
                                                   op0=Alu.mult, op1=Alu.add)

            def attn_chunk(s, ic, interleave=None, after_pair0=None):
                """Attention chunk pairs; returns the tail thunk (emitted by the
                caller after the NEXT chunk's first pair, to keep PE fed)."""
                Rcond = Rv[s] if ic == 0 else R2v[s]
                first = (ic == 0)
                ktt = kt_prefetch(s, ic, Rcond, first)
                psav = [psacc.tile([D + 1, ICW], f32, tag=f"psav{h}", name=f"psav{h}")
                        for h in range(H)]
                il = list(interleave or [])
                pTs = []

                def pairs_body():
                    for pr in range(NPAIR):
                        if first and pr == 0:
                            attn_pair(s, ic, pr, ktt, pTs, psav)
                        else:
                            with tc.If(Rcond > 2 * pr):
                                attn_pair(s, ic, pr, ktt, pTs, psav)
                        if pr == 0 and after_pair0 is not None:
                            after_pair0()
                        if il:
                            il.pop(0)()
                    while il:
                        il.pop(0)()

                pairs_body()

                def tail_thunk():
                    if first:
                        attn_tail(s, ic, psav, pTs)
                    else:
                        with tc.If(Rcond > 0):
                            attn_tail(s, ic, psav, pTs)
                return tail_thunk

            # ---------- MLP ----------
            h2T_keep = [None] * SPC
            mT_keep = [None] * SPC

            def ln2_block(s):
                h2T = big.tile([P, 2, N], bf16, tag="h2T", name=f"h2T{s}")
                h2T_keep[s] = h2T
                ln_transposed(x_sb[s], h2T)

            def mlp_fc1_gelu(s):
                h2T = h2T_keep[s]
                mT = mtp.tile([P, 8, N], bf16, tag="mT", name=f"mT{s}")
                mT_keep[s] = mT
                for r in range(8):
                    psf = psw.tile([P, 2, ICW], f32, tag="w", name="psf")
                    for icol in range(2):
                        for c2 in range(2):
                            nc.tensor.matmul(psf[:, icol, :],
                                             lhsT=wfc1(c2)[:, r * P:(r + 1) * P],
                                             rhs=h2T[:, c2, icol * ICW:(icol + 1) * ICW],
                                             start=(c2 == 0), stop=(c2 == 1))
                    nc.scalar.activation(out=mT[:, r, :].rearrange("p (a b) -> p a b", a=2),
                                         in_=psf[:], func=Act.Gelu,
                                         bias=bf1_sb[:, r:r + 1], scale=1.0)

            def mlp_fc2(s):
                mT = mT_keep[s]
                for t in range(NT):
                    psf2 = psw.tile([P, 2, ICW], f32, tag="w", name="psf2")
                    for r in range(8):
                        nc.tensor.matmul(psf2[:, 0, 0:C], lhsT=mT[:, r, t * P:(t + 1) * P],
                                         rhs=wfc2(r), start=(r == 0),
                                         stop=(r == 7 and not nz["f2b"]))
                    if nz["f2b"]:
                        nc.tensor.matmul(psf2[:, 0, 0:C], lhsT=ones1_bf[:], rhs=f2brow_sb,
                                         start=False, stop=True)
                    nc.vector.tensor_tensor(out=x_sb[s][:, t, :], in0=psf2[:, 0, 0:C],
                                            in1=x_sb[s][:, t, :], op=Alu.add)
                nc.sync.dma_start(out=out_ext[s].rearrange("(t p) c -> p t c", p=P),
                                  in_=x_sb[s][:])

            # ================= emission schedule =================
            # slot 0 = heavy sample (may have 2 query chunks), slot 1 = light
            # (always single-chunk; asserted host-side).
            hT0 = big.tile([P, 2, N], bf16, tag="hT", name="hT0")
            hT1 = big.tile([P, 2, N], bf16, tag="hT", name="hT1")

            # qkT chunk 0 of s0 is emitted mid-LN (needs only hT cols 0:512)
            ln_transposed(x_sb[0], hT0, mid_thunk=lambda: qkT_chunk(0, hT0, 0))
            if C_max > 1:
                with tc.If(R2v[0] > 0):
                    qkT_chunk(0, hT0, 1)
            va_block(0, hT0)

            def il_u_s0():
                uniform_block(0)

            def il_ln1_s1():
                ln_transposed(x_sb[1], hT1, mid_thunk=lambda: qkT_chunk(1, hT1, 0))

            def il_va_s1():
                va_block(1, hT1)

            def il_u_s1():
                uniform_block(1)

            t00 = attn_chunk(0, 0, interleave=[il_u_s0, il_ln1_s1, il_va_s1, il_u_s1])
            if C_max > 1:
                t01 = attn_chunk(0, 1, after_pair0=t00)
            else:
                t00()
                t01 = None

            def il_ln2_s0():
                ln2_block(0)

            t10 = attn_chunk(1, 0, interleave=[il_ln2_s0],
                             after_pair0=t01 if t01 is not None else None)
            t10()
            ln2_block(1)

            # all exp/ln ACT work is done -> single table swap to gelu
            mlp_fc1_gelu(0)
            mlp_fc2(0)
            mlp_fc1_gelu(1)
            mlp_fc2(1)

    nc.finalize()
    return nc


def _prep(inputs):
    """Host-side preprocessing: sharding metadata + weight folding + packing."""
    import ml_dtypes
    bf16 = ml_dtypes.bfloat16

    x = np.ascontiguousarray(np.asarray(inputs["x"], dtype=np.float32))
    K = np.asarray(inputs["K"], dtype=np.float32)
    n1 = np.asarray(inputs["n1"]).astype(np.int64)
    n2 = np.asarray(inputs["n2"]).astype(np.int64)
    nrc = n1 * n2
    scale = D ** -0.5

    g1 = np.asarray(inputs["ln1_g"], np.float32)
    b1 = np.asarray(inputs["ln1_b"], np.float32)
    g2 = np.asarray(inputs["ln2_g"], np.float32)
    b2 = np.asarray(inputs["ln2_b"], np.float32)
    qkv_w = np.asarray(inputs["qkv_w"], np.float32)
    qkv_b = np.asarray(inputs["qkv_b"], np.float32)

    Wqk = qkv_w[:2 * C]
    bqk = Wqk @ b1 + qkv_b[:2 * C]
    Wqk_eff = (Wqk * g1[None, :]).copy()
    Wqk_eff[:C] *= scale
    bqk = bqk.copy()
    bqk[:C] *= scale
    Wv = qkv_w[2 * C:]
    bv = Wv @ b1 + qkv_b[2 * C:]
    Wv_eff = Wv * g1[None, :]
    W1 = np.asarray(inputs["fc1_w"], np.float32)
    bf1 = W1 @ b2 + np.asarray(inputs["fc1_b"], np.float32)
    W1_eff = W1 * g2[None, :]
    pb = np.asarray(inputs["proj_b"], np.float32)
    f2b = np.asarray(inputs["fc2_b"], np.float32)

    nz = {
        "bqk": bool(np.any(bqk != 0)),
        "bv": bool(np.any(bv != 0)),
        "pb": bool(np.any(pb != 0)),
        "f2b": bool(np.any(f2b != 0)),
    }

    # --- bf16 weight blob [P, WB] ---
    WqkT = np.ascontiguousarray(Wqk_eff.T).astype(bf16)
    WvT = np.ascontiguousarray(Wv_eff.T).astype(bf16)
    ProjT = np.ascontiguousarray(np.asarray(inputs["proj_w"], np.float32).T).astype(bf16)
    Fc1T = np.ascontiguousarray(W1_eff.T).astype(bf16)
    Fc2T = np.ascontiguousarray(np.asarray(inputs["fc2_w"], np.float32).T).astype(bf16)
    wb = np.zeros((P, WB), bf16)
    for c2 in range(2):
        rows = slice(c2 * P, (c2 + 1) * P)
        o = c2 * WB_C2
        wb[:, o + O_WQK:o + O_WQK + 2 * C] = WqkT[rows]
        wb[:, o + O_WV:o + O_WV + C] = WvT[rows]
        wb[:, o + O_PROJ:o + O_PROJ + C] = ProjT[rows]
        wb[:, o + O_FC1:o + O_FC1 + 4 * C] = Fc1T[rows]
    wb[:, O_ID:O_ID + P] = np.eye(P, dtype=np.float32).astype(bf16)
    for r in range(8):
        wb[:, O_FC2 + r * C:O_FC2 + (r + 1) * C] = Fc2T[r * P:(r + 1) * P]
    wb[0, O_ROWS:O_ROWS + C] = bv.astype(bf16)
    wb[0, O_ROWS + C:O_ROWS + 2 * C] = pb.astype(bf16)
    wb[0, O_ROWS + 2 * C:O_ROWS + 3 * C] = f2b.astype(bf16)
    wb_f32 = np.ascontiguousarray(wb).view(np.float32)

    # --- f32 blob [P, F32W] ---
    cb = np.zeros((P, F32W), np.float32)
    cb[:, F_BQK:F_BQK + 4] = bqk.reshape(4, P).T
    cb[:, F_BV:F_BV + 2] = bv.reshape(2, P).T
    cb[:, F_BF1:F_BF1 + 8] = bf1.reshape(8, P).T
    cb[:, F_IOTA:F_IOTA + NT] = (np.arange(P, dtype=np.float32)[:, None]
                                 + P * np.arange(NT, dtype=np.float32)[None, :])
    sel4 = np.zeros((P, C), np.float32)
    for h in range(H):
        sel4[32 * h, h * D:(h + 1) * D] = 1.0
    cb[:, F_SEL:F_SEL + C] = sel4

    # balance: pair by attention tile cost
    Rc = ((nrc + P - 1) // P).astype(np.int64)
    Cc2 = ((nrc + ICW - 1) // ICW).astype(np.int64)
    cost = Rc * Cc2
    order = np.argsort(cost, kind="stable")
    pairs = [(int(order[B - 1 - i]), int(order[i])) for i in range(NCORES)]
    assert all(int(Cc2[b]) == 1 for _, b in pairs), "light slot must be single-chunk"

    # kt: K transposed, with invalid-key rows (j >= nrc) baked to NEG
    kt_all = np.ascontiguousarray(K.transpose(0, 2, 1))
    jidx = np.arange(N)
    for s in range(B):
        kt_all[s, jidx >= nrc[s], :] = NEG
    kt_all = kt_all.astype(bf16)

    in_maps = []
    for a, b in pairs:
        kt_pair = kt_all[[a, b]].reshape(SPC, -1).view(np.float32)
        x_pair = x[[a, b]].reshape(SPC, -1)
        bigv = np.ascontiguousarray(np.concatenate([kt_pair, x_pair], axis=1))
        meta = np.zeros(8, np.int32)
        for i, sidx in enumerate((a, b)):
            R_i = int(Rc[sidx])
            meta[2 * i] = R_i
            meta[2 * i + 1] = R_i if int(Cc2[sidx]) >= 2 else 0
            meta[6 + i] = (R_i + 1) // 2 - 1
        meta[4:6] = np.asarray(nrc[[a, b]], np.float32).view(np.int32)
        in_maps.append({
            "wb": wb_f32,
            "cb": cb,
            "meta": meta.reshape(1, 8),
            "big": bigv,
        })

    R_max = int(np.max(Rc))
    C_max = int(np.max(Cc2))
    return in_maps, pairs, R_max, C_max, nz


def kernel(**inputs):
    from concourse.bass_utils import run_bass_kernel_spmd

    in_maps, pairs, R_max, C_max, nz = _prep(inputs)
    nc = _build(R_max, C_max, nz)
    res = run_bass_kernel_spmd(nc, in_maps, core_ids=list(range(NCORES)), trace=False)

    out = np.empty((B, N, C), np.float32)
    for c, (a, b) in enumerate(pairs):
        got = res.results[c]["out"]
        out[a] = got[0]
        out[b] = got[1]
    return out


if __name__ == "__main__":
    import reference as R

    inp = {k: np.asarray(v) for k, v in R.setup_inputs().items()}
    got = kernel(**inp)
    exp = np.asarray(R.reference(**inp))
    rel = np.linalg.norm(got - exp) / np.linalg.norm(exp)
    print("Relative error:", rel)
